# revision 31
# baseline (speedup 1.0000x reference)
"""ClusterKLLoss Trainium2 kernel (8 NeuronCores, j-sharded, fp8 DoubleRow).

Math (from the reference):
  loss = CE(logits, arange(B), sum)/B, logits[i,j] = -kl[i,j]/T
  kl[i,j] = hneg[j] - Li[i].Q[j], Q = softmax(c_j), hneg[j] = sum Q logQ.
  Per-row-i shifts cancel in log-softmax, so with E = exp(c_j),
  Z_j = sum E, A_j = sum E*c_j, and T = 1/2:
    G[i,j] = (c_i[i].Q_j)/T + 2 lnZ_j - 2 A_j/Z_j   (logits + per-i const)
    loss   = sum_i [logsumexp_j G[i,j] - G[i,i]] / B

Sharding: core c owns c_j rows [512c, 512c+512) (4 partition-tiles of 128 j)
and the FULL c_i as a host-transposed fp8 ciT.  Each core computes stripes
S'[j=128, i=512] = sum_k W8[j,k] ciT[k,i] with W8 = E*(CW/Z_j) in fp8
(DoubleRow, K=256/matmul).  The scalar engine forms u = exp(S'/256 + bb_j)
with per-partition bias bb_j = -2A/Z - b0c (b0c keeps u ~1 in fp16), the
DVE scales by zn2_j = (Z_j/Z0)^2 and accumulates the 4 j-tiles into
zp[j=128, i-chunks] fp16.  Host sums zp over cores+partitions into Zi and
lse_i = ln Zi + b0c + 2 ln Z0.

ciT rides in an eighth-major host layout [1024, 8192] (row e*128+p, col
kt*512+i') ROLLED per core so local i-chunk 0 is the core's own j-range:
the diagonal G_ii comes straight out of chunk-0 stripes via an eye mask
(gd = diag(S')*SINV + bb_j = G_ii - b0c), all SPMD-uniform.  fp8 noise on
the diag is ~3e-3/entry and averages out (~5e-6 on the loss).

Constants b0c/Z0 cancel exactly in the host reduction.
"""

import sys

for _p in ("/opt/trn_rl_repo",):
    if _p not in sys.path:
        sys.path.insert(0, _p)

import numpy as np
import ml_dtypes

import concourse.bass as bass
import concourse.bacc as bacc
import concourse.tile as tile
from concourse import mybir
from concourse import bass_utils

B = 4096
D = 2048
TEMP = 0.5
NCORES = 8
SHARD = B // NCORES  # 512 j-rows per core
NT = SHARD // 128  # 4 j partition-tiles
KT = D // 128  # 16 k partition-tiles
NCH = 8  # i chunks of 512

F32 = mybir.dt.float32
F16 = mybir.dt.float16
F8 = mybir.dt.float8e4
AF = mybir.ActivationFunctionType
OP = mybir.AluOpType
AX = mybir.AxisListType
PM = mybir.MatmulPerfMode

CW = 512.0  # W8 = CW*Q: typ ~0.25, max ~15; fp8e4 max is 240
SINV = 2.0 / CW  # S' * SINV = (ci.Q)/T
B0C = -2.0  # bias recenter: u = exp(G - 2lnZ - B0C) stays ~1 in fp16
Z0 = 3400.0  # zn2 = (Z/Z0)^2 ~ 1


def build_kernel_body(tc, zp_ap, gd_ap, zo_ap, cit_ap, cj_ap, cjt_ap, eye_ap):
    nc = tc.nc
    from contextlib import ExitStack

    with ExitStack() as ctx:
        singles = ctx.enter_context(tc.tile_pool(name="singles", bufs=1))
        xin = ctx.enter_context(tc.tile_pool(name="xin", bufs=2))
        scr = ctx.enter_context(tc.tile_pool(name="scr", bufs=2))
        zqp = ctx.enter_context(tc.tile_pool(name="zqp", bufs=2))
        ups = ctx.enter_context(tc.tile_pool(name="ups", bufs=3))
        pps = ctx.enter_context(tc.tile_pool(name="pps", bufs=2))
        dps = ctx.enter_context(tc.tile_pool(name="dps", bufs=2))
        psS = ctx.enter_context(tc.tile_pool(name="psS", bufs=6, space="PSUM"))

        # resident tiles
        cit8 = singles.tile([128, KT, B], F8)  # 64KB/part
        E16s = singles.tile([128, NT, D], F16)  # 16KB/part (j-major, Z/A)
        cjt16 = singles.tile([128, NT, KT, 128], F16)  # 16KB/part (k-major)
        et16 = singles.tile([128, NT, KT, 128], F16)  # 16KB/part exp(cjT)
        wt8 = singles.tile([128, NT, KT, 128], F8)  # 8KB/part
        pacc = singles.tile([128, NCH, 512], F16)  # 8KB/part
        eye32 = singles.tile([128, 128], F32)
        Zc = singles.tile([128, NT], F32)
        Ac = singles.tile([128, NT], F32)
        rzc = singles.tile([128, NT], F32)
        scw = singles.tile([128, NT], F32)
        bb2 = singles.tile([128, NT], F32)
        zn2 = singles.tile([128, NT], F32)
        tmp1 = singles.tile([128, NT], F32)
        gd = singles.tile([128, NT], F32)

        # ── loads ─────────────────────────────────────────────────────────
        # gpsimd software-DGE DMAs are fire-and-forget (engine issues the
        # descriptors and moves on), so ALL bulk traffic rides gpsimd: the 8
        # ciT eighths, cj1-3, eye and later the outputs.  The sync (hwdge)
        # ring carries only the PE-critical chain: cj0 halves + transposes.
        cj_tiles = [None] * NT

        def load_cj(eng, t):
            cj_tiles[t] = xin.tile([128, D], F16, tag="cj", name=f"cjt{t}", bufs=4)
            if t == 0:
                for h in range(2):
                    eng.dma_start(
                        out=cj_tiles[t][:, 1024 * h : 1024 * (h + 1)],
                        in_=cj_ap[0:128, 1024 * h : 1024 * (h + 1)],
                    )
            else:
                eng.dma_start(
                    out=cj_tiles[t], in_=cj_ap[128 * t : 128 * (t + 1), :]
                )

        def load_e(e):
            nc.sync.dma_start(
                out=cit8[:, :, 512 * e : 512 * (e + 1)],
                in_=cit_ap[128 * e : 128 * (e + 1), :],
            )

        # deterministic sync-ring order (all dep-free => priority order):
        # all cj tiles first (so every prep is scheduler-ready before any
        # stripe work), then the 8 ciT eighths
        nc.sync.dma_start(out=cjt16[:, 0], in_=cjt_ap[0:128, 0:2048])
        load_e(0)
        for t in range(1, NT):
            nc.sync.dma_start(
                out=cjt16[:, t], in_=cjt_ap[0:128, 2048 * t : 2048 * (t + 1)]
            )
        load_cj(nc.sync, 0)
        load_e(1)
        load_cj(nc.sync, 1)
        load_e(2)
        load_cj(nc.sync, 2)
        load_e(3)
        load_cj(nc.sync, 3)
        for e in range(4, NCH):
            load_e(e)
        nc.gpsimd.dma_start(out=eye32, in_=eye_ap)

        # ── prep for all 4 j-tiles (emitted before any stripes) ───────────
        # critical chain per 1024-col half: exp -> transpose -> fp8 cast.
        # The per-j softmax scale 2/Z_j rides the stripe activation's
        # per-partition scale AP.  NB: [128,1] tensor_tensor is a 4.6us
        # degenerate slow path on DVE -- use tensor_scalar with AP scalars.
        # k-major critical chain: exp(cjT) chunk -> fp8 cast = the weights
        # for j-tile t.  No device transpose anywhere.
        for t in range(NT):
            nc.scalar.activation(
                out=et16[:, t], in_=cjt16[:, t], func=AF.Exp
            )
            nc.vector.tensor_copy(out=wt8[:, t], in_=et16[:, t])

        # j-major pipeline (off the PE-critical path): Z, A, biases
        for t in range(NT):
            cj_t = cj_tiles[t]
            E16 = E16s[:, t]
            nc.scalar.activation(
                out=E16, in_=cj_t, func=AF.Exp, accum_out=Zc[:, t : t + 1]
            )
            nc.vector.reciprocal(out=rzc[:, t : t + 1], in_=Zc[:, t : t + 1])
            nc.vector.tensor_scalar_mul(
                scw[:, t : t + 1], rzc[:, t : t + 1], 2.0
            )
            j1 = scr.tile([128, D], F16, tag="j1")
            nc.gpsimd.tensor_mul(j1, E16, cj_t)
            nc.vector.tensor_reduce(
                out=Ac[:, t : t + 1], in_=j1, axis=AX.X, op=OP.add
            )
            nc.vector.tensor_scalar_mul(
                tmp1[:, t : t + 1], rzc[:, t : t + 1], -2.0
            )
            nc.vector.tensor_scalar(
                out=bb2[:, t : t + 1], in0=Ac[:, t : t + 1],
                scalar1=tmp1[:, t : t + 1], scalar2=-float(B0C),
                op0=OP.mult, op1=OP.add,
            )
            nc.vector.tensor_scalar_mul(
                tmp1[:, t : t + 1], Zc[:, t : t + 1], float(1.0 / Z0)
            )
            nc.vector.tensor_scalar_mul(
                zn2[:, t : t + 1], tmp1[:, t : t + 1], tmp1[:, t : t + 1]
            )

        # ── stripes: S'[j=128, i=512]; chunks 0-3 for all t, then 4-7 ─────
        for crange in (range(0, NCH // 2), range(NCH // 2, NCH)):
            for t in range(NT):
                for c2 in crange:
                    S_ps = psS.tile([128, 512], F32, tag="s")
                    for k2 in range(KT // 2):
                        nc.tensor.matmul(
                            S_ps,
                            wt8[:, t, 2 * k2 : 2 * k2 + 2, :],
                            cit8[:, 2 * k2 : 2 * k2 + 2, 512 * c2 : 512 * (c2 + 1)],
                            start=(k2 == 0),
                            stop=(k2 == KT // 2 - 1),
                            perf_mode=PM.DoubleRow,
                        )
                    if c2 == 0:
                        # diagonal: G_ii - b0c = diag(S')*SINV + bb2
                        junk = dps.tile([128, 128], F32, tag="junk")
                        nc.vector.tensor_mul(
                            junk, S_ps[:, 128 * t : 128 * (t + 1)], eye32
                        )
                        dd = dps.tile([128, 1], F32, tag="dd")
                        nc.vector.tensor_reduce(
                            out=dd, in_=junk, axis=AX.X, op=OP.add
                        )
                        nc.vector.tensor_scalar(
                            out=gd[:, t : t + 1], in0=dd,
                            scalar1=scw[:, t : t + 1], scalar2=bb2[:, t : t + 1],
                            op0=OP.mult, op1=OP.add,
                        )
                    u16 = ups.tile([128, 512], F16, tag="u16")
                    nc.scalar.activation(
                        out=u16, in_=S_ps, func=AF.Exp,
                        scale=scw[:, t : t + 1], bias=bb2[:, t : t + 1],
                    )
                    if t == 0:
                        nc.vector.tensor_scalar_mul(
                            pacc[:, c2], u16, zn2[:, t : t + 1]
                        )
                    else:
                        p16 = pps.tile([128, 512], F16, tag="p16")
                        nc.vector.tensor_scalar_mul(p16, u16, zn2[:, t : t + 1])
                        nc.vector.tensor_add(pacc[:, c2], pacc[:, c2], p16)
                    if t == NT - 1:
                        nc.sync.dma_start(
                            out=zp_ap[:, 512 * c2 : 512 * (c2 + 1)],
                            in_=pacc[:, c2],
                        )

        nc.sync.dma_start(out=gd_ap, in_=gd)
        nc.sync.dma_start(out=zo_ap, in_=Zc)


_NC_CACHE = {}


def build_nc():
    key = "nc"
    if key in _NC_CACHE:
        return _NC_CACHE[key]
    nc = bacc.Bacc("TRN2", target_bir_lowering=False, debug=False)
    cit = nc.dram_tensor("cit", [NCH * 128, KT * 512], F8, kind="ExternalInput").ap()
    cj = nc.dram_tensor("cj", [SHARD, D], F16, kind="ExternalInput").ap()
    cjt = nc.dram_tensor("cjt", [128, KT * 512], F16, kind="ExternalInput").ap()
    eye = nc.dram_tensor("eye", [128, 128], F32, kind="ExternalInput").ap()
    zp = nc.dram_tensor("zp", [128, B], F16, kind="ExternalOutput").ap()
    gd = nc.dram_tensor("gd", [128, NT], F32, kind="ExternalOutput").ap()
    zo = nc.dram_tensor("zo", [128, NT], F32, kind="ExternalOutput").ap()
    with tile.TileContext(nc) as tc:
        build_kernel_body(tc, zp, gd, zo, cit, cj, cjt, eye)
    nc.compile()
    _NC_CACHE[key] = nc
    return nc


def make_in_maps(c_i, c_j):
    # eighth-major ciT: block e is rows [512e, 512e+512) of c_i as columns;
    # per core, roll blocks so local chunk 0 = the core's own j-range (the
    # diagonal block).
    cit8 = c_i.T.astype(ml_dtypes.float8_e4m3)  # [D, B] = [(kt p), (e i')]
    base = np.ascontiguousarray(
        cit8.reshape(KT, 128, NCH, 512).transpose(2, 1, 0, 3)
    )  # [e, p, kt, i']
    eye = np.eye(128, dtype=np.float32)
    in_maps = []
    for c in range(NCORES):
        rolled = np.ascontiguousarray(np.roll(base, -c, axis=0)).reshape(
            NCH * 128, KT * 512
        )
        cjsh = c_j[SHARD * c : SHARD * (c + 1)].astype(np.float16)
        in_maps.append(
            {
                "cit": rolled,
                "cj": cjsh,
                "cjt": np.ascontiguousarray(
                    cjsh.T.reshape(KT, 128, NT, 128).transpose(1, 2, 0, 3)
                ).reshape(128, KT * SHARD),
                "eye": eye,
            }
        )
    return in_maps


def kernel(c_i, c_j, **kwargs):
    c_i = np.ascontiguousarray(np.asarray(c_i, dtype=np.float32))
    c_j = np.ascontiguousarray(np.asarray(c_j, dtype=np.float32))
    nc = build_nc()
    in_maps = make_in_maps(c_i, c_j)
    res = bass_utils.run_bass_kernel_spmd(
        nc, in_maps, core_ids=list(range(NCORES))
    )

    Zi = np.zeros(B, dtype=np.float64)
    gii_sum = np.float64(0.0)
    for c, r in enumerate(res.results):
        zl = r["zp"].astype(np.float64).sum(axis=0).reshape(NCH, 512)
        Zi += np.roll(zl, c, axis=0).reshape(-1)
        # G_ii = gd + 2 lnZ + b0c  (gd = S*SINV + bb2 lacks the 2 lnZ term)
        gii_sum += (
            r["gd"].astype(np.float64)
            + 2.0 * np.log(r["zo"].astype(np.float64))
            + B0C
        ).sum()
    lse_sum = np.log(Zi).sum() + B * (B0C + 2.0 * np.log(Z0))
    loss = (lse_sum - gii_sum) / B
    return np.float32(loss).reshape(())


# revision 32
# speedup vs baseline: 1.1372x; 1.1372x over previous
"""ClusterKLLoss Trainium2 kernel (8 NeuronCores, j-sharded, fp8 DoubleRow).

Math (from the reference):
  loss = CE(logits, arange(B), sum)/B, logits[i,j] = -kl[i,j]/T
  kl[i,j] = hneg[j] - Li[i].Q[j], Q = softmax(c_j), hneg[j] = sum Q logQ.
  Per-row-i shifts cancel in log-softmax, so with E = exp(c_j),
  Z_j = sum E, A_j = sum E*c_j, and T = 1/2:
    G[i,j] = (c_i[i].Q_j)/T + 2 lnZ_j - 2 A_j/Z_j   (logits + per-i const)
    loss   = sum_i [logsumexp_j G[i,j] - G[i,i]] / B

Sharding: core c owns c_j rows [512c, 512c+512) (4 partition-tiles of 128 j)
and the FULL c_i as a host-transposed fp8 ciT.  Each core computes stripes
S'[j=128, i=512] = sum_k W8[j,k] ciT[k,i] with W8 = E*(CW/Z_j) in fp8
(DoubleRow, K=256/matmul).  The scalar engine forms u = exp(S'/256 + bb_j)
with per-partition bias bb_j = -2A/Z - b0c (b0c keeps u ~1 in fp16), the
DVE scales by zn2_j = (Z_j/Z0)^2 and accumulates the 4 j-tiles into
zp[j=128, i-chunks] fp16.  Host sums zp over cores+partitions into Zi and
lse_i = ln Zi + b0c + 2 ln Z0.

ciT rides in an eighth-major host layout [1024, 8192] (row e*128+p, col
kt*512+i') ROLLED per core so local i-chunk 0 is the core's own j-range:
the diagonal G_ii comes straight out of chunk-0 stripes via an eye mask
(gd = diag(S')*SINV + bb_j = G_ii - b0c), all SPMD-uniform.  fp8 noise on
the diag is ~3e-3/entry and averages out (~5e-6 on the loss).

Constants b0c/Z0 cancel exactly in the host reduction.
"""

import sys

for _p in ("/opt/trn_rl_repo",):
    if _p not in sys.path:
        sys.path.insert(0, _p)

import numpy as np
import ml_dtypes

import concourse.bass as bass
import concourse.bacc as bacc
import concourse.tile as tile
from concourse import mybir
from concourse import bass_utils

B = 4096
D = 2048
TEMP = 0.5
NCORES = 8
SHARD = B // NCORES  # 512 j-rows per core
NT = SHARD // 128  # 4 j partition-tiles
KT = D // 128  # 16 k partition-tiles
NCH = 8  # i chunks of 512

F32 = mybir.dt.float32
F16 = mybir.dt.float16
F8 = mybir.dt.float8e4
AF = mybir.ActivationFunctionType
OP = mybir.AluOpType
AX = mybir.AxisListType
PM = mybir.MatmulPerfMode

CW = 512.0  # W8 = CW*Q: typ ~0.25, max ~15; fp8e4 max is 240
SINV = 2.0 / CW  # S' * SINV = (ci.Q)/T
B0C = -2.0  # bias recenter: u = exp(G - 2lnZ - B0C) stays ~1 in fp16
Z0 = 3400.0  # zn2 = (Z/Z0)^2 ~ 1


def build_kernel_body(tc, zp_ap, gd_ap, zo_ap, cit_ap, cj_ap, cjt_ap, eye_ap):
    nc = tc.nc
    from contextlib import ExitStack

    with ExitStack() as ctx:
        singles = ctx.enter_context(tc.tile_pool(name="singles", bufs=1))
        xin = ctx.enter_context(tc.tile_pool(name="xin", bufs=2))
        scr = ctx.enter_context(tc.tile_pool(name="scr", bufs=2))
        zqp = ctx.enter_context(tc.tile_pool(name="zqp", bufs=2))
        ups = ctx.enter_context(tc.tile_pool(name="ups", bufs=3))
        pps = ctx.enter_context(tc.tile_pool(name="pps", bufs=2))
        dps = ctx.enter_context(tc.tile_pool(name="dps", bufs=2))
        psS = ctx.enter_context(tc.tile_pool(name="psS", bufs=6, space="PSUM"))

        # resident tiles
        cit8 = singles.tile([128, KT, B], F8)  # 64KB/part
        E16s = singles.tile([128, NT, D], F16)  # 16KB/part (j-major, Z/A)
        cjt16 = singles.tile([128, KT, SHARD], F16)  # 16KB/part (k-major)
        et16 = singles.tile([128, KT, SHARD], F16)  # 16KB/part exp(cjT)
        wt8 = singles.tile([128, KT, SHARD], F8)  # 8KB/part
        pacc = singles.tile([128, NCH, 512], F16)  # 8KB/part
        eye32 = singles.tile([128, 128], F32)
        Zc = singles.tile([128, NT], F32)
        Ac = singles.tile([128, NT], F32)
        rzc = singles.tile([128, NT], F32)
        scw = singles.tile([128, NT], F32)
        bb2 = singles.tile([128, NT], F32)
        zn2 = singles.tile([128, NT], F32)
        tmp1 = singles.tile([128, NT], F32)
        gd = singles.tile([128, NT], F32)

        # ── loads ─────────────────────────────────────────────────────────
        # gpsimd software-DGE DMAs are fire-and-forget (engine issues the
        # descriptors and moves on), so ALL bulk traffic rides gpsimd: the 8
        # ciT eighths, cj1-3, eye and later the outputs.  The sync (hwdge)
        # ring carries only the PE-critical chain: cj0 halves + transposes.
        cj_tiles = [None] * NT

        def load_cj(eng, t):
            cj_tiles[t] = xin.tile([128, D], F16, tag="cj", name=f"cjt{t}", bufs=4)
            if t == 0:
                for h in range(2):
                    eng.dma_start(
                        out=cj_tiles[t][:, 1024 * h : 1024 * (h + 1)],
                        in_=cj_ap[0:128, 1024 * h : 1024 * (h + 1)],
                    )
            else:
                eng.dma_start(
                    out=cj_tiles[t], in_=cj_ap[128 * t : 128 * (t + 1), :]
                )

        def load_e(e):
            nc.sync.dma_start(
                out=cit8[:, :, 512 * e : 512 * (e + 1)],
                in_=cit_ap[128 * e : 128 * (e + 1), :],
            )

        # deterministic sync-ring order (all dep-free => priority order):
        # all cj tiles first (so every prep is scheduler-ready before any
        # stripe work), then the 8 ciT eighths
        nc.sync.dma_start(out=cjt16, in_=cjt_ap)
        for t in range(NT):
            load_cj(nc.sync, t)
        for e in range(NCH):
            load_e(e)
        nc.gpsimd.dma_start(out=eye32, in_=eye_ap)

        # ── prep for all 4 j-tiles (emitted before any stripes) ───────────
        # critical chain per 1024-col half: exp -> transpose -> fp8 cast.
        # The per-j softmax scale 2/Z_j rides the stripe activation's
        # per-partition scale AP.  NB: [128,1] tensor_tensor is a 4.6us
        # degenerate slow path on DVE -- use tensor_scalar with AP scalars.
        # k-major critical chain: exp(cjT) chunk -> fp8 cast = the weights
        # for j-tile t.  No device transpose anywhere.
        for t in range(NT):
            nc.scalar.activation(
                out=et16[:, :, 128 * t : 128 * (t + 1)],
                in_=cjt16[:, :, 128 * t : 128 * (t + 1)],
                func=AF.Exp,
            )
            nc.vector.tensor_copy(
                out=wt8[:, :, 128 * t : 128 * (t + 1)],
                in_=et16[:, :, 128 * t : 128 * (t + 1)],
            )

        # j-major pipeline (off the PE-critical path): Z, A, biases
        for t in range(NT):
            cj_t = cj_tiles[t]
            E16 = E16s[:, t]
            nc.scalar.activation(
                out=E16, in_=cj_t, func=AF.Exp, accum_out=Zc[:, t : t + 1]
            )
            nc.vector.reciprocal(out=rzc[:, t : t + 1], in_=Zc[:, t : t + 1])
            nc.vector.tensor_scalar_mul(
                scw[:, t : t + 1], rzc[:, t : t + 1], 2.0
            )
            j1 = scr.tile([128, D], F16, tag="j1")
            nc.gpsimd.tensor_mul(j1, E16, cj_t)
            nc.vector.tensor_reduce(
                out=Ac[:, t : t + 1], in_=j1, axis=AX.X, op=OP.add
            )
            nc.vector.tensor_scalar_mul(
                tmp1[:, t : t + 1], rzc[:, t : t + 1], -2.0
            )
            nc.vector.tensor_scalar(
                out=bb2[:, t : t + 1], in0=Ac[:, t : t + 1],
                scalar1=tmp1[:, t : t + 1], scalar2=-float(B0C),
                op0=OP.mult, op1=OP.add,
            )
            nc.vector.tensor_scalar_mul(
                tmp1[:, t : t + 1], Zc[:, t : t + 1], float(1.0 / Z0)
            )
            nc.vector.tensor_scalar_mul(
                zn2[:, t : t + 1], tmp1[:, t : t + 1], tmp1[:, t : t + 1]
            )

        # ── stripes: S'[j=128, i=512]; chunks 0-3 for all t, then 4-7 ─────
        for crange in (range(0, NCH // 2), range(NCH // 2, NCH)):
            for t in range(NT):
                for c2 in crange:
                    S_ps = psS.tile([128, 512], F32, tag="s")
                    for k2 in range(KT // 2):
                        nc.tensor.matmul(
                            S_ps,
                            wt8[:, 2 * k2 : 2 * k2 + 2, 128 * t : 128 * (t + 1)],
                            cit8[:, 2 * k2 : 2 * k2 + 2, 512 * c2 : 512 * (c2 + 1)],
                            start=(k2 == 0),
                            stop=(k2 == KT // 2 - 1),
                            perf_mode=PM.DoubleRow,
                        )
                    if c2 == 0:
                        # diagonal: G_ii - b0c = diag(S')*SINV + bb2
                        junk = dps.tile([128, 128], F32, tag="junk")
                        nc.vector.tensor_mul(
                            junk, S_ps[:, 128 * t : 128 * (t + 1)], eye32
                        )
                        dd = dps.tile([128, 1], F32, tag="dd")
                        nc.vector.tensor_reduce(
                            out=dd, in_=junk, axis=AX.X, op=OP.add
                        )
                        nc.vector.tensor_scalar(
                            out=gd[:, t : t + 1], in0=dd,
                            scalar1=scw[:, t : t + 1], scalar2=bb2[:, t : t + 1],
                            op0=OP.mult, op1=OP.add,
                        )
                    u16 = ups.tile([128, 512], F16, tag="u16")
                    nc.scalar.activation(
                        out=u16, in_=S_ps, func=AF.Exp,
                        scale=scw[:, t : t + 1], bias=bb2[:, t : t + 1],
                    )
                    if t == 0:
                        nc.vector.tensor_scalar_mul(
                            pacc[:, c2], u16, zn2[:, t : t + 1]
                        )
                    else:
                        p16 = pps.tile([128, 512], F16, tag="p16")
                        nc.vector.tensor_scalar_mul(p16, u16, zn2[:, t : t + 1])
                        nc.vector.tensor_add(pacc[:, c2], pacc[:, c2], p16)
                    if t == NT - 1:
                        nc.sync.dma_start(
                            out=zp_ap[:, 512 * c2 : 512 * (c2 + 1)],
                            in_=pacc[:, c2],
                        )

        nc.sync.dma_start(out=gd_ap, in_=gd)
        nc.sync.dma_start(out=zo_ap, in_=Zc)


_NC_CACHE = {}


def build_nc():
    key = "nc"
    if key in _NC_CACHE:
        return _NC_CACHE[key]
    nc = bacc.Bacc("TRN2", target_bir_lowering=False, debug=False)
    cit = nc.dram_tensor("cit", [NCH * 128, KT * 512], F8, kind="ExternalInput").ap()
    cj = nc.dram_tensor("cj", [SHARD, D], F16, kind="ExternalInput").ap()
    cjt = nc.dram_tensor("cjt", [128, KT * 512], F16, kind="ExternalInput").ap()
    eye = nc.dram_tensor("eye", [128, 128], F32, kind="ExternalInput").ap()
    zp = nc.dram_tensor("zp", [128, B], F16, kind="ExternalOutput").ap()
    gd = nc.dram_tensor("gd", [128, NT], F32, kind="ExternalOutput").ap()
    zo = nc.dram_tensor("zo", [128, NT], F32, kind="ExternalOutput").ap()
    with tile.TileContext(nc) as tc:
        build_kernel_body(tc, zp, gd, zo, cit, cj, cjt, eye)
    nc.compile()
    _NC_CACHE[key] = nc
    return nc


def make_in_maps(c_i, c_j):
    # eighth-major ciT: block e is rows [512e, 512e+512) of c_i as columns;
    # per core, roll blocks so local chunk 0 = the core's own j-range (the
    # diagonal block).
    cit8 = c_i.T.astype(ml_dtypes.float8_e4m3)  # [D, B] = [(kt p), (e i')]
    base = np.ascontiguousarray(
        cit8.reshape(KT, 128, NCH, 512).transpose(2, 1, 0, 3)
    )  # [e, p, kt, i']
    eye = np.eye(128, dtype=np.float32)
    in_maps = []
    for c in range(NCORES):
        rolled = np.ascontiguousarray(np.roll(base, -c, axis=0)).reshape(
            NCH * 128, KT * 512
        )
        cjsh = c_j[SHARD * c : SHARD * (c + 1)].astype(np.float16)
        in_maps.append(
            {
                "cit": rolled,
                "cj": cjsh,
                "cjt": np.ascontiguousarray(
                    cjsh.T.reshape(KT, 128, SHARD).transpose(1, 0, 2)
                ).reshape(128, KT * SHARD),
                "eye": eye,
            }
        )
    return in_maps


def kernel(c_i, c_j, **kwargs):
    c_i = np.ascontiguousarray(np.asarray(c_i, dtype=np.float32))
    c_j = np.ascontiguousarray(np.asarray(c_j, dtype=np.float32))
    nc = build_nc()
    in_maps = make_in_maps(c_i, c_j)
    res = bass_utils.run_bass_kernel_spmd(
        nc, in_maps, core_ids=list(range(NCORES))
    )

    Zi = np.zeros(B, dtype=np.float64)
    gii_sum = np.float64(0.0)
    for c, r in enumerate(res.results):
        zl = r["zp"].astype(np.float64).sum(axis=0).reshape(NCH, 512)
        Zi += np.roll(zl, c, axis=0).reshape(-1)
        # G_ii = gd + 2 lnZ + b0c  (gd = S*SINV + bb2 lacks the 2 lnZ term)
        gii_sum += (
            r["gd"].astype(np.float64)
            + 2.0 * np.log(r["zo"].astype(np.float64))
            + B0C
        ).sum()
    lse_sum = np.log(Zi).sum() + B * (B0C + 2.0 * np.log(Z0))
    loss = (lse_sum - gii_sum) / B
    return np.float32(loss).reshape(())
